# revision 1
# baseline (speedup 1.0000x reference)
"""Distributed kNN OOD-score kernel for 8 Trainium2 NeuronCores.

Problem: for each of 4*32*32 query vectors (D=768), find the 3 nearest
database vectors (N=20000, squared-L2), average the 3 distances, and
bilinearly upsample the resulting [4,32,32] map to [4,1,512,512].

Sharding: queries are data-parallel. Each core owns half of one batch
image (16 of 32 query rows = 512 queries); the database is replicated
and streamed through SBUF in bf16. The one halo row each core needs for
the 16x bilinear upsample is exchanged with its pair core via a tiny
AllGather. Each core computes the 4-row block its PAIR needs first
(local tile 0), so the AllGather launches ~40us before the matmul
stream ends and its ~15us latency is fully hidden. The per-core
interpolation matrix (host input) absorbs the resulting row permutation,
keeping the device program SPMD-uniform.

Per-core device program:
  - scores t[q,n] = q.x - ||x||^2/2 via TensorE: 6 bf16 K=128 matmuls
    (stationary query tile, moving db columns) + one K=2 matmul adding
    the -||x||^2/2 row in split-bf16 (hi+lo) precision, accumulated f32
    in PSUM.
  - ScalarE evacuates each 500-col PSUM bank into an SBUF score strip.
  - VectorE max8 per [128,4000] strip -> per-strip top-8; final max8
    over the 40 strip winners -> global top-3 per query (values only).
  - mean distance = reduce_sum of sqrt((q^2 - 2t)/9) (ScalarE fused
    scale+bias+sqrt).
  - pair AllGather of local tile 0's 128 ood values (boundary block).
  - 16x bilinear upsample = two small f32 matmuls with interpolation
    matrices (built on host; verified against jax.image.resize).
"""

import sys

if "/opt/trn_rl_repo" not in sys.path:
    sys.path.insert(0, "/opt/trn_rl_repo")

import numpy as np
import ml_dtypes

import concourse.bass as bass
import concourse.bacc as bacc
import concourse.mybir as mybir
import concourse.tile as tile
from concourse import bass_utils

# Problem shape (hardcoded per contract).
B, D, H, W = 4, 768, 32, 32
N = 20000
K_NN = 3
OUT_H = OUT_W = 512
N_CORES = 8

SC = 4000           # db columns per super-chunk (8 PSUM banks * 500)
N_SC = N // SC      # 5
BANK = 500
N_BANK = SC // BANK  # 8
HALF = SC // 2      # db DMA granularity (finer for startup overlap)
KC = D // 128       # 6 contraction chunks
QPC = 512           # queries scored per core (16 rows)
N_QT = QPC // 128   # 4
OROWS = 256         # output rows per core
NCOL = 24           # ood columns entering the upsample (16 own + 2x4 gathered)

F32 = mybir.dt.float32
BF16 = mybir.dt.bfloat16
AX = mybir.AxisListType
AF = mybir.ActivationFunctionType

# local tile -> 4-row block of this core's half (block i = rows 4i..4i+3).
# Tile 0 is the block the PAIR core needs as its halo row: for the top
# half (rows 0-15) that's block 3 (row 15), for the bottom half (rows
# 16-31) block 0 (row 16).
TILE_BLOCKS = ([3, 0, 1, 2], [0, 1, 2, 3])


def _build_program():
    nc = bacc.Bacc(
        "TRN2", target_bir_lowering=False, debug=False, num_devices=N_CORES
    )
    dbT = nc.dram_tensor("dbT", [D, N], BF16, kind="ExternalInput").ap()
    xh = nc.dram_tensor("xh", [2, N], BF16, kind="ExternalInput").ap()
    qT = nc.dram_tensor("qT", [128, KC * QPC], BF16, kind="ExternalInput").ap()
    q2 = nc.dram_tensor("q2", [128, N_QT], F32, kind="ExternalInput").ap()
    art = nc.dram_tensor("art", [NCOL, OROWS], F32, kind="ExternalInput").ap()
    ac = nc.dram_tensor("ac", [W, OUT_W], F32, kind="ExternalInput").ap()
    out = nc.dram_tensor("out", [OROWS, OUT_W], F32, kind="ExternalOutput").ap()

    with tile.TileContext(nc) as tc:
        with (
            tc.tile_pool(name="static", bufs=1) as sp,
            tc.tile_pool(name="db", bufs=4 * KC) as dbp,
            tc.tile_pool(name="scores", bufs=2) as scp,
            tc.tile_pool(name="small", bufs=4) as smp,
            tc.tile_pool(name="psum", bufs=N_BANK, space="PSUM") as pp,
            tc.tile_pool(name="dram", bufs=1, space="DRAM") as dp,
        ):
            # queries per k-chunk so the first matmul only waits for k=0;
            # first super-chunk's db tiles loaded before everything else
            qk_sb = [
                sp.tile([128, QPC], BF16, name=f"qk{k}") for k in range(KC)
            ]
            db0_tiles = []
            for k in range(KC):
                nc.sync.dma_start(qk_sb[k][:], qT[:, k * QPC : (k + 1) * QPC])
                for h in range(2):
                    t = dbp.tile([128, HALF], BF16, tag="db", name=f"db0_{k}_{h}")
                    nc.sync.dma_start(
                        t[:], dbT[k * 128 : (k + 1) * 128, h * HALF : (h + 1) * HALF]
                    )
                    db0_tiles.append(t)
            xh_sb = sp.tile([2, N], BF16)
            nc.sync.dma_start(xh_sb[:], xh[:])
            q2_sb = sp.tile([128, N_QT], F32)
            nc.sync.dma_start(q2_sb[:], q2[:])
            art_sb = sp.tile([NCOL, OROWS], F32)
            nc.sync.dma_start(art_sb[:], art[:])
            ac_sb = sp.tile([W, OUT_W], F32)
            nc.sync.dma_start(ac_sb[:], ac[:])
            ones2 = sp.tile([2, 128], BF16)
            nc.gpsimd.memset(ones2[:], 1.0)

            # per-query-tile top-8 winners of each (super-chunk, bank)
            parts = [
                sp.tile([128, N_SC * N_BANK * 8], F32, name=f"part{qt}")
                for qt in range(N_QT)
            ]
            # qt 0 separate so the collective only depends on it
            oods = [
                sp.tile([128, 1], F32, name=f"ood{qt}") for qt in range(N_QT)
            ]
            cc_in = dp.tile([128], F32)
            cc_out = dp.tile([256], F32)
            scratch = dp.tile([QPC], F32)

            for sc in range(N_SC):
                if sc == 0:
                    db_tiles = db0_tiles
                else:
                    db_tiles = []
                    for k in range(KC):
                        for h in range(2):
                            t = dbp.tile(
                                [128, HALF], BF16, tag="db", name=f"db{k}_{h}"
                            )
                            nc.sync.dma_start(
                                t[:],
                                dbT[
                                    k * 128 : (k + 1) * 128,
                                    sc * SC + h * HALF : sc * SC + (h + 1) * HALF,
                                ],
                            )
                            db_tiles.append(t)
                for qt in range(N_QT):
                    bank_sb = [
                        scp.tile([128, BANK], F32, tag=f"sb{b}", name=f"sb{b}")
                        for b in range(N_BANK)
                    ]
                    banks = [
                        pp.tile([128, BANK], F32, tag="bank", name=f"bank{b}")
                        for b in range(N_BANK)
                    ]
                    for k in range(KC):
                        lhsT = qk_sb[k][:, qt * 128 : (qt + 1) * 128]
                        for b in range(N_BANK):
                            src = db_tiles[2 * k + (b * BANK) // HALF]
                            off = (b * BANK) % HALF
                            nc.tensor.matmul(
                                banks[b][:],
                                lhsT,
                                src[:, off : off + BANK],
                                start=(k == 0),
                                stop=False,
                            )
                    for b in range(N_BANK):
                        nc.tensor.matmul(
                            banks[b][:],
                            ones2[:],
                            xh_sb[:, sc * SC + b * BANK : sc * SC + (b + 1) * BANK],
                            start=False,
                            stop=True,
                        )
                    for b in range(N_BANK):
                        nc.scalar.activation(bank_sb[b][:], banks[b][:], AF.Copy)
                        nc.vector.max(
                            parts[qt][
                                :, (sc * N_BANK + b) * 8 : (sc * N_BANK + b + 1) * 8
                            ],
                            bank_sb[b][:],
                        )

                    if sc != N_SC - 1:
                        continue
                    # epilogue inline after this qt's last strip (engines run
                    # their queues in order — emitting it later would trap it
                    # behind the remaining strip maxes)
                    f8 = smp.tile([128, 8], F32, tag="f8")
                    nc.vector.max(f8[:], parts[qt][:])
                    # dist_j/3 = sqrt((q2 - 2 t_j) / 9); host passes q2/9
                    d3 = smp.tile([128, K_NN], F32, tag="d3")
                    nc.scalar.activation(
                        d3[:],
                        f8[:, 0:K_NN],
                        AF.Sqrt,
                        bias=q2_sb[:, qt : qt + 1],
                        scale=-2.0 / 9.0,
                    )
                    nc.vector.reduce_sum(oods[qt][:], d3[:], axis=AX.X)
                    if qt == 0:
                        # boundary block: gather it across the pair ASAP so
                        # the ~15us collective hides under remaining matmuls
                        nc.sync.dma_start(cc_in[:], oods[0][:])
                        nc.gpsimd.collective_compute(
                            "AllGather",
                            mybir.AluOpType.bypass,
                            replica_groups=[[0, 1], [2, 3], [4, 5], [6, 7]],
                            ins=[cc_in.opt()],
                            outs=[cc_out.opt()],
                        )
                    # own ood values -> scratch incrementally (local order)
                    nc.sync.dma_start(
                        scratch.rearrange("(q p) -> p q", p=128)[:, qt : qt + 1],
                        oods[qt][:],
                    )

            # ood_hT[c, j]: j 0..15 own rows (local order), 16..23 the two
            # gathered boundary blocks in rank order
            ood_hT = sp.tile([W, NCOL], F32)
            nc.sync.dma_start(
                ood_hT[:, 0:16], scratch.rearrange("(r c) -> c r", c=W)
            )
            nc.sync.dma_start(
                ood_hT[:, 16:NCOL],
                cc_out.rearrange("(b r c) -> c (b r)", b=2, c=W),
            )

            # P1[j, ow] = sum_c ood_hT[c, j] * A_c[c, ow]
            p1 = pp.tile([NCOL, OUT_W], F32, tag="bank")
            nc.tensor.matmul(p1[:], ood_hT[:], ac_sb[:], start=True, stop=True)
            p1_sb = sp.tile([NCOL, OUT_W], F32)
            nc.scalar.activation(p1_sb[:], p1[:], AF.Copy)
            # out[oi, ow] = sum_j art[j, oi] * P1[j, ow]
            for m in range(2):
                p2 = pp.tile([128, OUT_W], F32, tag="bank", name=f"p2_{m}")
                nc.tensor.matmul(
                    p2[:],
                    art_sb[:, m * 128 : (m + 1) * 128],
                    p1_sb[:],
                    start=True,
                    stop=True,
                )
                o_sb = smp.tile([128, OUT_W], F32, tag="osb", name=f"osb{m}")
                nc.scalar.activation(o_sb[:], p2[:], AF.Copy)
                nc.sync.dma_start(out[m * 128 : (m + 1) * 128, :], o_sb[:])

    nc.compile()
    return nc


def _bilinear_matrix(out_size: int, in_size: int) -> np.ndarray:
    """Half-pixel (align_corners=False) bilinear interpolation matrix
    [out_size, in_size]; edge-clamped, equivalent to jax.image.resize
    'bilinear' for integer upsampling."""
    A = np.zeros((out_size, in_size), dtype=np.float64)
    scale = in_size / out_size
    for i in range(out_size):
        s = (i + 0.5) * scale - 0.5
        j0 = int(np.floor(s))
        w = s - j0
        A[i, min(max(j0, 0), in_size - 1)] += 1.0 - w
        A[i, min(max(j0 + 1, 0), in_size - 1)] += w
    return A.astype(np.float32)


_NC_CACHE = None


def _get_nc():
    global _NC_CACHE
    if _NC_CACHE is None:
        _NC_CACHE = _build_program()
    return _NC_CACHE


def make_in_maps(embeddings: np.ndarray, database: np.ndarray):
    embeddings = np.asarray(embeddings, dtype=np.float32)
    database = np.asarray(database, dtype=np.float32)

    dbT = np.ascontiguousarray(database.T).astype(ml_dtypes.bfloat16)
    # -||x||^2/2 in split bf16 (hi + lo)
    xh_f = -0.5 * np.einsum("nd,nd->n", database, database)
    hi = xh_f.astype(ml_dtypes.bfloat16)
    lo = (xh_f - hi.astype(np.float32)).astype(ml_dtypes.bfloat16)
    xh = np.stack([hi, lo])

    q_all = embeddings.transpose(0, 2, 3, 1).reshape(B, H * W, D)
    Ac = _bilinear_matrix(OUT_W, W)                      # [512, 32]
    Ar = _bilinear_matrix(OUT_H, H)                      # [512, 32]
    # the two gathered blocks in cc_out rank order: pair-core tile 0 rows
    cc_rows = [12, 13, 14, 15, 16, 17, 18, 19]

    in_maps = []
    for c in range(N_CORES):
        b, half = divmod(c, 2)
        blocks = TILE_BLOCKS[half]
        own_rows = [16 * half + 4 * blk + r for blk in blocks for r in range(4)]

        # queries in local-tile order
        q = np.concatenate(
            [
                q_all[b, (16 * half + 4 * blk) * W : (16 * half + 4 * blk + 4) * W]
                for blk in blocks
            ]
        )                                                # [512, 768]
        qTb = (
            np.ascontiguousarray(q.T)                    # [768, 512]
            .reshape(KC, 128, QPC)
            .transpose(1, 0, 2)
            .reshape(128, KC * QPC)
            .astype(ml_dtypes.bfloat16)
        )
        q2 = np.einsum("qd,qd->q", q, q) / 9.0
        q2 = np.ascontiguousarray(q2.reshape(N_QT, 128).T.astype(np.float32))

        # interpolation rows matching ood_hT's column order
        Arh = Ar[half * OROWS : (half + 1) * OROWS]      # [256, 32]
        art = np.zeros((NCOL, OROWS), dtype=np.float32)
        for j, row in enumerate(own_rows):
            art[j] = Arh[:, row]
        for j, row in enumerate(cc_rows):
            if row not in own_rows:
                art[16 + j] = Arh[:, row]
        in_maps.append(
            {
                "dbT": dbT,
                "xh": xh,
                "qT": qTb,
                "q2": q2,
                "art": art,
                "ac": np.ascontiguousarray(Ac.T),        # [32, 512]
            }
        )
    return in_maps


def run_device(in_maps, **kwargs):
    nc = _get_nc()
    return bass_utils.run_bass_kernel_spmd(
        nc, in_maps, core_ids=list(range(N_CORES)), **kwargs
    )


def kernel(embeddings, database, k, out_h, out_w):
    assert int(k) == K_NN and int(out_h) == OUT_H and int(out_w) == OUT_W
    in_maps = make_in_maps(np.asarray(embeddings), np.asarray(database))
    res = run_device(in_maps)
    out = np.empty((B, 1, OUT_H, OUT_W), dtype=np.float32)
    for c in range(N_CORES):
        b, half = divmod(c, 2)
        out[b, 0, half * OROWS : (half + 1) * OROWS] = res.results[c]["out"]
    return out



# revision 2
# speedup vs baseline: 2.5394x; 2.5394x over previous
"""Distributed kNN OOD-score kernel for 8 Trainium2 NeuronCores.

Problem: for each of 4*32*32 query vectors (D=768), find the 3 nearest
database vectors (N=20000, squared-L2), average the 3 distances, and
bilinearly upsample the resulting [4,32,32] map to [4,1,512,512].

Sharding: queries are data-parallel. Each core owns half of one batch
image (16 of 32 query rows = 512 queries); the database is replicated
and streamed through SBUF in fp8-e4m3. The one halo row each core needs
for the 16x bilinear upsample is exchanged with its pair core via a tiny
AllGather (the per-core interpolation matrix absorbs the resulting row
permutation, keeping the device program SPMD-uniform).

Scoring: t[q,n] = q.x - ||x||^2/2 via fp8 DoubleRow matmuls (K=256 per
pass, 3 passes, 2x column rate = 4x bf16 throughput). The contraction
carries 766 real dims plus two "slot" rows holding -||x||^2/2 in
split-fp8 (query-side scales 2 and 1), so the norm term rides along in
band; the two dropped embedding dims only lose their cross terms
(zero-mean noise ~2.8 in d^2, irrelevant at rel-err 2e-2).

Selection: top-3 per query = max over the (negated-distance-monotone)
scores. PSUM is organized as 4 rotating [128,2,512] f32 tiles (two
hardware banks each; matmul outputs must not straddle the 2KB banks).
Each 1000-col tile is drained by one of two paths, statically assigned
to balance engine load:
  - DVE direct: windowed tensor_reduce(max, win 20) PSUM -> bf16 strip.
  - Act evac: ScalarE copies the tile to bf16 SBUF; DVE then runs a
    win-8 tensor_max tree in the 2-byte 2x mode (two levels inline per
    group, final level at qt end).
A final DVE max8 over each query-tile's ~2050-col strip yields the top-3
scores; ScalarE turns them into distances (fused sqrt((q^2-2t)/9)) and
DVE reduce_sum averages them. GPSIMD (Pool) cannot touch PSUM nor run
TensorTensor on TRN2, so it only runs memsets and the collective.

Upsample: 16x bilinear = two small f32 matmuls with host-built
interpolation matrices (verified against jax.image.resize).
"""

import sys

if "/opt/trn_rl_repo" not in sys.path:
    sys.path.insert(0, "/opt/trn_rl_repo")

import numpy as np
import ml_dtypes

import concourse.bass as bass
import concourse.bacc as bacc
import concourse.mybir as mybir
import concourse.tile as tile
from concourse import bass_utils

# Problem shape (hardcoded per contract).
B, D, H, W = 4, 768, 32, 32
N = 20000
K_NN = 3
OUT_H = OUT_W = 512
N_CORES = 8

QPC = 512            # queries scored per core (16 rows)
N_QT = QPC // 128    # 4 query tiles
PASSES = 3           # fp8 DoubleRow K=256 contraction passes
DREAL = 766          # real embedding dims carried (dims 766,767 -> slots)
GCOLS = 2000         # db columns per DMA group
N_G = N // GCOLS     # 10
UCOLS = 1000         # db columns per PSUM tile (drain unit)
N_U = N // UCOLS     # 20 units per query tile
OROWS = 256          # output rows per core
NCOL = 24            # ood columns entering the upsample (16 own + 2x4 gathered)

# drain-path assignment: units (of 20 per qt) drained by DVE directly;
# the rest are evacuated by ScalarE and tree-maxed on DVE.
DVE_UNITS = (0, 3, 7, 10, 13, 17)
N_DVE = len(DVE_UNITS)           # 6
N_ACT = N_U - N_DVE              # 14
DWIN = 20                        # DVE direct window -> 50 strip cols/unit
DSTRIP = UCOLS // DWIN           # 50
ASTRIP = UCOLS // 8              # 125 strip cols per Act unit (win-8 tree)
STRIPW = N_DVE * DSTRIP + N_ACT * ASTRIP  # 2050

F32 = mybir.dt.float32
BF16 = mybir.dt.bfloat16
FP8 = mybir.dt.float8e4
E4M3 = ml_dtypes.float8_e4m3
AX = mybir.AxisListType
AF = mybir.ActivationFunctionType
DR = mybir.MatmulPerfMode.DoubleRow

# local tile -> 4-row block of this core's half (block i = rows 4i..4i+3).
# Tile 0 is the block the PAIR core needs as its halo row: for the top
# half (rows 0-15) that's block 3 (row 15), for the bottom half (rows
# 16-31) block 0 (row 16).
TILE_BLOCKS = ([3, 0, 1, 2], [0, 1, 2, 3])


def _unit_kind(u):
    return "dve" if u in DVE_UNITS else "act"


# strip column offset for each unit, plus tmp250 slot offsets for act units
_strip_off = {}
_tmp_off = {}
_d_i = 0
_a_i = 0
for _u in range(N_U):
    if _unit_kind(_u) == "dve":
        _strip_off[_u] = _d_i * DSTRIP
        _d_i += 1
    else:
        _strip_off[_u] = N_DVE * DSTRIP + _a_i * ASTRIP
        _tmp_off[_u] = _a_i * 250
        _a_i += 1


def _build_program():
    nc = bacc.Bacc(
        "TRN2", target_bir_lowering=False, debug=False, num_devices=N_CORES
    )
    dbx = nc.dram_tensor("dbx", [D, N], FP8, kind="ExternalInput").ap()
    qx = nc.dram_tensor(
        "qx", [128, PASSES, N_QT, 2, 128], FP8, kind="ExternalInput"
    ).ap()
    q2 = nc.dram_tensor("q2", [128, N_QT], F32, kind="ExternalInput").ap()
    art = nc.dram_tensor("art", [NCOL, OROWS], F32, kind="ExternalInput").ap()
    ac = nc.dram_tensor("ac", [W, OUT_W], F32, kind="ExternalInput").ap()
    out = nc.dram_tensor("out", [OROWS, OUT_W], F32, kind="ExternalOutput").ap()

    with tile.TileContext(nc) as tc:
        with (
            tc.tile_pool(name="static", bufs=1) as sp,
            tc.tile_pool(name="db", bufs=3 * PASSES) as dbp,
            tc.tile_pool(name="evac", bufs=6) as evp,
            tc.tile_pool(name="tmp5", bufs=3) as t5p,
            tc.tile_pool(name="small", bufs=4) as smp,
            tc.tile_pool(name="psum", bufs=4, space="PSUM") as pp,
            tc.tile_pool(name="dram", bufs=1, space="DRAM") as dp,
        ):
            q_sb = sp.tile([128, PASSES, N_QT, 2, 128], FP8)
            nc.sync.dma_start(q_sb[:], qx[:])
            q2_sb = sp.tile([128, N_QT], F32)
            nc.sync.dma_start(q2_sb[:], q2[:])
            art_sb = sp.tile([NCOL, OROWS], F32)
            nc.sync.dma_start(art_sb[:], art[:])
            ac_sb = sp.tile([W, OUT_W], F32)
            nc.sync.dma_start(ac_sb[:], ac[:])

            # PE p-state warmup: junk DoubleRow matmuls keep the PE busy
            # from t~0 so the ramp (0.65/1.2GHz for the first ~3us of
            # activity) burns while the first db tiles are still in
            # flight, not on the real score stream.
            junk = sp.tile([128, 2, 512], FP8)
            nc.gpsimd.memset(junk[:], 0.0)
            for wu in range(12):
                wt = pp.tile([128, 2, 512], F32, tag="ps", name="warm")
                nc.tensor.matmul(
                    wt[:, 0, :], q_sb[:, 0, 0], junk[:], start=True, stop=True,
                    perf_mode=DR,
                )

            # per-qt bf16 strip of window-max scores + tree scratch
            strips = [
                sp.tile([128, STRIPW], BF16, name=f"strip{qt}")
                for qt in range(N_QT)
            ]
            tmp250 = [
                sp.tile([128, N_ACT * 250], BF16, name=f"t250_{qt}")
                for qt in range(N_QT)
            ]
            oods = [
                sp.tile([128, 1], F32, name=f"ood{qt}") for qt in range(N_QT)
            ]
            cc_in = dp.tile([128], F32)
            cc_out = dp.tile([256], F32)
            scratch = dp.tile([QPC], F32)

            # stream the db by column group; first group's tiles load first
            db_tiles = {}
            for g in range(N_G):
                for j in range(PASSES):
                    t = dbp.tile([128, 2, GCOLS], FP8, tag="db", name=f"db{g}_{j}")
                    nc.sync.dma_start(
                        t[:],
                        dbx[
                            256 * j : 256 * (j + 1),
                            g * GCOLS : (g + 1) * GCOLS,
                        ].rearrange("(i p) c -> p i c", i=2),
                    )
                    db_tiles[(g, j)] = t

            def qt_end(qt):
                # tree level 3: [128, N_ACT, 250] -> strip [128, N_ACT, 125]
                nc.vector.tensor_max(
                    strips[qt][:, N_DVE * DSTRIP : STRIPW].rearrange(
                        "p (a c) -> p a c", c=ASTRIP
                    ),
                    tmp250[qt][:].rearrange("p (a c) -> p a c", c=250)[:, :, 0:125],
                    tmp250[qt][:].rearrange("p (a c) -> p a c", c=250)[:, :, 125:250],
                )
                f8 = smp.tile([128, 8], BF16, tag="f8", name="f8")
                nc.vector.max(f8[:], strips[qt][:])
                # dist_j/3 = sqrt((q2 - 2 t_j)/9); host passes q2/9
                d3 = smp.tile([128, K_NN], F32, tag="d3", name="d3")
                nc.scalar.activation(
                    d3[:],
                    f8[:, 0:K_NN],
                    AF.Sqrt,
                    bias=q2_sb[:, qt : qt + 1],
                    scale=-2.0 / 9.0,
                )
                nc.vector.reduce_sum(oods[qt][:], d3[:], axis=AX.X)
                if qt == 0:
                    # boundary block: gather it across the pair ASAP so
                    # the collective hides under the remaining tail
                    nc.sync.dma_start(cc_in[:], oods[0][:])
                    nc.gpsimd.collective_compute(
                        "AllGather",
                        mybir.AluOpType.bypass,
                        replica_groups=[[0, 1], [2, 3], [4, 5], [6, 7]],
                        ins=[cc_in.opt()],
                        outs=[cc_out.opt()],
                    )
                nc.sync.dma_start(
                    scratch.rearrange("(q p) -> p q", p=128)[:, qt : qt + 1],
                    oods[qt][:],
                )

            for g in range(N_G):
                for qt in range(N_QT):
                    for h in range(2):
                        u = g * 2 + h
                        ps = pp.tile([128, 2, 512], F32, tag="ps", name="ps")
                        for j in range(PASSES):
                            for bk in range(2):
                                c0 = h * UCOLS + bk * 500
                                nc.tensor.matmul(
                                    ps[:, bk, 0:500],
                                    q_sb[:, j, qt],
                                    db_tiles[(g, j)][:, :, c0 : c0 + 500],
                                    start=(j == 0),
                                    stop=(j == PASSES - 1),
                                    perf_mode=DR,
                                )
                        so = _strip_off[u]
                        if _unit_kind(u) == "dve":
                            nc.vector.tensor_reduce(
                                strips[qt][:, so : so + DSTRIP],
                                ps[:, :, 0:500].rearrange(
                                    "p b (w k) -> p b w k", k=DWIN
                                ),
                                axis=AX.X,
                                op=mybir.AluOpType.max,
                            )
                        else:
                            ev = evp.tile([128, UCOLS], BF16, tag="ev", name="ev")
                            nc.scalar.activation(ev[:], ps[:, :, 0:500], AF.Copy)
                            t5 = t5p.tile([128, 500], BF16, tag="t5", name="t5")
                            nc.vector.tensor_max(
                                t5[:], ev[:, 0:500], ev[:, 500:1000]
                            )
                            to = _tmp_off[u]
                            nc.vector.tensor_max(
                                tmp250[qt][:, to : to + 250],
                                t5[:, 0:250],
                                t5[:, 250:500],
                            )
                    if g == N_G - 1:
                        qt_end(qt)

            # ood_hT[c, j]: j 0..15 own rows (local order), 16..23 the two
            # gathered boundary blocks in rank order
            ood_hT = sp.tile([W, NCOL], F32)
            nc.sync.dma_start(
                ood_hT[:, 0:16], scratch.rearrange("(r c) -> c r", c=W)
            )
            nc.sync.dma_start(
                ood_hT[:, 16:NCOL],
                cc_out.rearrange("(b r c) -> c (b r)", b=2, c=W),
            )

            # P1[j, ow] = sum_c ood_hT[c, j] * A_c[c, ow]
            p1 = pp.tile([NCOL, OUT_W], F32, tag="ps", name="p1")
            nc.tensor.matmul(p1[:], ood_hT[:], ac_sb[:], start=True, stop=True)
            p1_sb = sp.tile([NCOL, OUT_W], F32)
            nc.scalar.activation(p1_sb[:], p1[:], AF.Copy)
            # out[oi, ow] = sum_j art[j, oi] * P1[j, ow]
            for m in range(2):
                p2 = pp.tile([128, OUT_W], F32, tag="ps", name=f"p2_{m}")
                nc.tensor.matmul(
                    p2[:],
                    art_sb[:, m * 128 : (m + 1) * 128],
                    p1_sb[:],
                    start=True,
                    stop=True,
                )
                o_sb = smp.tile([128, OUT_W], F32, tag="osb", name=f"osb{m}")
                nc.scalar.activation(o_sb[:], p2[:], AF.Copy)
                nc.sync.dma_start(out[m * 128 : (m + 1) * 128, :], o_sb[:])

    nc.compile()
    return nc


def _bilinear_matrix(out_size: int, in_size: int) -> np.ndarray:
    """Half-pixel (align_corners=False) bilinear interpolation matrix
    [out_size, in_size]; edge-clamped, equivalent to jax.image.resize
    'bilinear' for integer upsampling."""
    A = np.zeros((out_size, in_size), dtype=np.float64)
    scale = in_size / out_size
    for i in range(out_size):
        s = (i + 0.5) * scale - 0.5
        j0 = int(np.floor(s))
        w = s - j0
        A[i, min(max(j0, 0), in_size - 1)] += 1.0 - w
        A[i, min(max(j0 + 1, 0), in_size - 1)] += w
    return A.astype(np.float32)


_NC_CACHE = None


def _get_nc():
    global _NC_CACHE
    if _NC_CACHE is None:
        _NC_CACHE = _build_program()
    return _NC_CACHE


def make_in_maps(embeddings: np.ndarray, database: np.ndarray):
    embeddings = np.asarray(embeddings, dtype=np.float32)
    database = np.asarray(database, dtype=np.float32)

    # db fp8 layout: rows 0..765 = dims, rows 766/767 = -||x||^2/2 in
    # split fp8 with query-side scales (2, 1)
    r = -0.5 * np.einsum("nd,nd->n", database, database)
    slotA = (r / 2.0).astype(E4M3)
    slotB = (r - 2.0 * slotA.astype(np.float32)).astype(E4M3)
    dbx = np.empty((D, N), dtype=E4M3)
    dbx[0:DREAL] = np.ascontiguousarray(database.T[0:DREAL]).astype(E4M3)
    dbx[DREAL] = slotA
    dbx[DREAL + 1] = slotB

    q_all = embeddings.transpose(0, 2, 3, 1).reshape(B, H * W, D)
    Ac = _bilinear_matrix(OUT_W, W)                      # [512, 32]
    Ar = _bilinear_matrix(OUT_H, H)                      # [512, 32]
    # the two gathered blocks in cc_out rank order: pair-core tile 0 rows
    cc_rows = [12, 13, 14, 15, 16, 17, 18, 19]

    in_maps = []
    for c in range(N_CORES):
        b, half = divmod(c, 2)
        blocks = TILE_BLOCKS[half]
        own_rows = [16 * half + 4 * blk + r_ for blk in blocks for r_ in range(4)]

        # queries in local-tile order
        q = np.concatenate(
            [
                q_all[b, (16 * half + 4 * blk) * W : (16 * half + 4 * blk + 4) * W]
                for blk in blocks
            ]
        )                                                # [512, 768]
        Qx = np.empty((D, QPC), dtype=E4M3)
        Qx[0:DREAL] = np.ascontiguousarray(q.T[0:DREAL]).astype(E4M3)
        Qx[DREAL] = 2.0
        Qx[DREAL + 1] = 1.0
        # device layout qx[p, j, qt, i, m] = Qx[256j + 128i + p, 128qt + m]
        qxb = np.ascontiguousarray(
            Qx.reshape(PASSES, 2, 128, N_QT, 128).transpose(2, 0, 3, 1, 4)
        )
        q2 = np.einsum("qd,qd->q", q, q) / 9.0
        q2 = np.ascontiguousarray(q2.reshape(N_QT, 128).T.astype(np.float32))

        # interpolation rows matching ood_hT's column order
        Arh = Ar[half * OROWS : (half + 1) * OROWS]      # [256, 32]
        art = np.zeros((NCOL, OROWS), dtype=np.float32)
        for j, row in enumerate(own_rows):
            art[j] = Arh[:, row]
        for j, row in enumerate(cc_rows):
            if row not in own_rows:
                art[16 + j] = Arh[:, row]
        in_maps.append(
            {
                "dbx": dbx,
                "qx": qxb,
                "q2": q2,
                "art": art,
                "ac": np.ascontiguousarray(Ac.T),        # [32, 512]
            }
        )
    return in_maps


def run_device(in_maps, **kwargs):
    nc = _get_nc()
    return bass_utils.run_bass_kernel_spmd(
        nc, in_maps, core_ids=list(range(N_CORES)), **kwargs
    )


def kernel(embeddings, database, k, out_h, out_w):
    assert int(k) == K_NN and int(out_h) == OUT_H and int(out_w) == OUT_W
    in_maps = make_in_maps(np.asarray(embeddings), np.asarray(database))
    res = run_device(in_maps)
    out = np.empty((B, 1, OUT_H, OUT_W), dtype=np.float32)
    for c in range(N_CORES):
        b, half = divmod(c, 2)
        out[b, 0, half * OROWS : (half + 1) * OROWS] = res.results[c]["out"]
    return out


# revision 5
# speedup vs baseline: 2.6667x; 1.0502x over previous
"""Distributed kNN OOD-score kernel for 8 Trainium2 NeuronCores.

Problem: for each of 4*32*32 query vectors (D=768), find the 3 nearest
database vectors (N=20000, squared-L2), average the 3 distances, and
bilinearly upsample the resulting [4,32,32] map to [4,1,512,512].

Sharding: queries are data-parallel. Each core owns half of one batch
image (16 of 32 query rows = 512 queries); the database is replicated
and streamed through SBUF in fp8-e4m3. The one halo row each core needs
for the 16x bilinear upsample is exchanged with its pair core via a tiny
AllGather (the per-core interpolation matrix absorbs the resulting row
permutation, keeping the device program SPMD-uniform).

Scoring: t[q,n] = q.x - ||x||^2/2 via fp8 DoubleRow matmuls (K=256 per
pass, 3 passes, 2x column rate = 4x bf16 throughput). The contraction
carries 766 real dims plus two "slot" rows holding -||x||^2/2 in
split-fp8 (query-side scales 2 and 1), so the norm term rides along in
band; the two dropped embedding dims only lose their cross terms
(zero-mean noise ~2.8 in d^2, irrelevant at rel-err 2e-2).

Selection: top-3 per query = max over the (negated-distance-monotone)
scores. PSUM holds 4 rotating [128,2,512] f32 tiles (two hardware banks
each; matmul outputs must not straddle the 2KB banks). Each 1000-col
tile is drained by one of two statically assigned paths:
  - DVE direct: windowed tensor_reduce(max, win 20) PSUM -> bf16 strip.
  - Act evac: ScalarE copies the tile to bf16 SBUF; DVE runs a win-8
    tensor_max tree in the 2-byte 2x mode (two wide levels per group,
    final level at qt end).
A final DVE max8 per query tile yields the top-3 scores; ScalarE turns
them into distances (fused sqrt((q^2-2t)/9)) and DVE reduce_sum
averages. qt0 (the halo block the pair core needs) runs a split max8 -
its partial top-8 is folded at group 3 - so its ood finishes right
after the last matmul group and the pair AllGather's ~15us latency
hides entirely under the other query tiles' selection tails. GPSIMD
cannot touch PSUM nor run TensorTensor on TRN2, so it only runs memsets
and the collective.

Upsample: 16x bilinear = two small float32r matmuls (1 cycle/row) with
host-built interpolation matrices, split into an own-rows part
(pre-computed before the collective lands) and the 8 gathered columns.
"""

import sys

if "/opt/trn_rl_repo" not in sys.path:
    sys.path.insert(0, "/opt/trn_rl_repo")

import numpy as np
import ml_dtypes

import concourse.bass as bass
import concourse.bacc as bacc
import concourse.mybir as mybir
import concourse.tile as tile
from concourse import bass_utils

# Problem shape (hardcoded per contract).
B, D, H, W = 4, 768, 32, 32
N = 20000
K_NN = 3
OUT_H = OUT_W = 512
N_CORES = 8

QPC = 512            # queries scored per core (16 rows)
N_QT = QPC // 128    # 4 query tiles
PASSES = 3           # fp8 DoubleRow K=256 contraction passes
DREAL = 766          # real embedding dims carried (dims 766,767 -> slots)
GCOLS = 4000         # db columns per DMA group
N_G = N // GCOLS     # 5
UCOLS = 1000         # db columns per PSUM tile (drain unit)
UPG = GCOLS // UCOLS  # 4 units per group
N_U = N // UCOLS     # 20 units per query tile
OROWS = 256          # output rows per core
NCOL = 24            # ood columns entering the upsample (16 own + 2x4 gathered)

# drain-path assignment: units (of 20 per qt) drained by DVE directly;
# the rest are evacuated by ScalarE and tree-maxed on DVE.
DVE_UNITS = (0, 3, 7, 10, 13, 17)
DWIN = 20                        # DVE direct window -> 50 strip cols/unit
DSTRIP = UCOLS // DWIN           # 50
ASTRIP = UCOLS // 8              # 125 strip cols per Act unit (win-8 tree)

F32 = mybir.dt.float32
F32R = mybir.dt.float32r
BF16 = mybir.dt.bfloat16
FP8 = mybir.dt.float8e4
E4M3 = ml_dtypes.float8_e4m3
AX = mybir.AxisListType
AF = mybir.ActivationFunctionType
DR = mybir.MatmulPerfMode.DoubleRow
MAX = mybir.AluOpType.max

# local tile -> 4-row block of this core's half (block i = rows 4i..4i+3).
# Tile 0 is the block the PAIR core needs as its halo row: for the top
# half (rows 0-15) that's block 3 (row 15), for the bottom half (rows
# 16-31) block 0 (row 16).
TILE_BLOCKS = ([3, 0, 1, 2], [0, 1, 2, 3])

# --- static unit bookkeeping ------------------------------------------------
# Units 0..15 land in groups 0-3 ("A" phase), 16..19 in group 4 ("B").
DVE_A = [u for u in range(16) if u in DVE_UNITS]          # 5 units
ACT_A = [u for u in range(16) if u not in DVE_UNITS]      # 11 units
DVE_B = [u for u in range(16, N_U) if u in DVE_UNITS]     # 1 unit
ACT_B = [u for u in range(16, N_U) if u not in DVE_UNITS]  # 3 units
N_ACT = len(ACT_A) + len(ACT_B)                            # 14

# strip layouts. qt0 gets an extra 8-col slot between the A and B
# regions holding the partial (group 0-3) top-8, so its final max8 only
# scans the group-4 tail.
#   qt0:   [dveA 250][actA 1375][t8 8][dveB 50][actB 375]      W=2058
#   qt1-3: [dveA 250][actA 1375][dveB 50][actB 375]            W=2050
W_A = len(DVE_A) * DSTRIP + len(ACT_A) * ASTRIP            # 1625


def _strip_layout(qt):
    off = {}
    p = 0
    for u in DVE_A:
        off[u] = p
        p += DSTRIP
    a_off = {}
    for i, u in enumerate(ACT_A):
        off[u] = p + i * ASTRIP
    p += len(ACT_A) * ASTRIP
    t8 = None
    if qt == 0:
        t8 = p
        p += 8
    for u in DVE_B:
        off[u] = p
        p += DSTRIP
    actb = p
    for i, u in enumerate(ACT_B):
        off[u] = p + i * ASTRIP
    p += len(ACT_B) * ASTRIP
    return off, t8, actb, p


_TMP_OFF = {u: i * 250 for i, u in enumerate(ACT_A + ACT_B)}


def _build_program():
    nc = bacc.Bacc(
        "TRN2", target_bir_lowering=False, debug=False, num_devices=N_CORES
    )
    dbx = nc.dram_tensor("dbx", [D, N], FP8, kind="ExternalInput").ap()
    qx = nc.dram_tensor(
        "qx", [128, PASSES, N_QT, 2, 128], FP8, kind="ExternalInput"
    ).ap()
    q2 = nc.dram_tensor("q2", [128, N_QT], F32, kind="ExternalInput").ap()
    art = nc.dram_tensor("art", [NCOL, OROWS], BF16, kind="ExternalInput").ap()
    ac = nc.dram_tensor("ac", [W, OUT_W], BF16, kind="ExternalInput").ap()
    out = nc.dram_tensor("out", [OROWS, OUT_W], F32, kind="ExternalOutput").ap()

    layouts = [_strip_layout(qt) for qt in range(N_QT)]

    with tile.TileContext(nc) as tc:
        with (
            tc.tile_pool(name="static", bufs=1) as sp,
            tc.tile_pool(name="db", bufs=6) as dbp,
            tc.tile_pool(name="evac", bufs=6) as evp,
            tc.tile_pool(name="tmp5", bufs=3) as t5p,
            tc.tile_pool(name="small", bufs=4) as smp,
            tc.tile_pool(name="psum", bufs=4, space="PSUM") as pp,
            tc.tile_pool(name="dram", bufs=1, space="DRAM") as dp,
        ):
            q_sb = sp.tile([128, PASSES, N_QT, 2, 128], FP8)
            nc.sync.dma_start(q_sb[:], qx[:])
            q2_sb = sp.tile([128, N_QT], F32)
            nc.sync.dma_start(q2_sb[:], q2[:])
            art_a = sp.tile([16, OROWS], BF16)
            nc.sync.dma_start(art_a[:], art[0:16, :])
            art_b = sp.tile([8, OROWS], BF16)
            nc.sync.dma_start(art_b[:], art[16:NCOL, :])
            ac_sb = sp.tile([W, OUT_W], BF16)
            nc.sync.dma_start(ac_sb[:], ac[:])

            # PE p-state warmup on junk data (no input dependencies), and
            # a dummy Sqrt to pull the activation-table load off the
            # critical path of qt0's distance epilogue.
            junkq = sp.tile([128, 2, 128], FP8)
            nc.gpsimd.memset(junkq[:], 0.0)
            junk = sp.tile([128, 2, 512], FP8)
            nc.gpsimd.memset(junk[:], 0.0)
            warm1 = sp.tile([128, 1], F32)
            nc.gpsimd.memset(warm1[:], 1.0)
            warm_o = sp.tile([128, 1], F32)
            nc.scalar.activation(warm_o[:], warm1[:], AF.Sqrt)
            nc.scalar.activation(warm_o[:], warm1[:], AF.Copy)
            for wu in range(12):
                wt = pp.tile([128, 2, 512], F32, tag="ps", name="warm")
                nc.tensor.matmul(
                    wt[:, 0, :], junkq[:], junk[:], start=True, stop=True,
                    perf_mode=DR,
                )

            strips = [
                sp.tile([128, layouts[qt][3]], BF16, name=f"strip{qt}")
                for qt in range(N_QT)
            ]
            tmp250 = [
                sp.tile([128, N_ACT * 250], BF16, name=f"t250_{qt}")
                for qt in range(N_QT)
            ]
            oods = [
                sp.tile([128, 1], F32, name=f"ood{qt}") for qt in range(N_QT)
            ]
            cc_in = dp.tile([128], F32)
            cc_out = dp.tile([256], F32)
            scratch = dp.tile([QPC], F32)

            # stream the db by column group; first group's tiles load first
            db_tiles = {}
            for g in range(N_G):
                for j in range(PASSES):
                    t = dbp.tile([128, 2, GCOLS], FP8, tag="db", name=f"db{g}_{j}")
                    nc.sync.dma_start(
                        t[:],
                        dbx[
                            256 * j : 256 * (j + 1),
                            g * GCOLS : (g + 1) * GCOLS,
                        ].rearrange("(i p) c -> p i c", i=2),
                    )
                    db_tiles[(g, j)] = t

            ood_hT = sp.tile([W, NCOL], F32)

            def qt_end(qt):
                """Emit qt's final selection + distance epilogue."""
                off, t8, actb, wid = layouts[qt]
                tm = tmp250[qt][:].rearrange("p (a c) -> p a c", c=250)
                st = strips[qt]
                na, nb = len(ACT_A), len(ACT_B)
                if qt == 0:
                    # only the B-phase level 3 + tail max8 (A was folded
                    # into the t8 slot at group 3)
                    nc.vector.tensor_max(
                        st[:, actb : actb + nb * ASTRIP].rearrange(
                            "p (a c) -> p a c", c=ASTRIP
                        ),
                        tm[:, na : na + nb, 0:125],
                        tm[:, na : na + nb, 125:250],
                    )
                    m8_in = st[:, t8:wid]
                else:
                    nc.vector.tensor_max(
                        st[:, off[ACT_A[0]] : off[ACT_A[0]] + na * ASTRIP]
                        .rearrange("p (a c) -> p a c", c=ASTRIP),
                        tm[:, 0:na, 0:125],
                        tm[:, 0:na, 125:250],
                    )
                    nc.vector.tensor_max(
                        st[:, actb : actb + nb * ASTRIP].rearrange(
                            "p (a c) -> p a c", c=ASTRIP
                        ),
                        tm[:, na : na + nb, 0:125],
                        tm[:, na : na + nb, 125:250],
                    )
                    m8_in = st[:]
                f8 = smp.tile([128, 8], BF16, tag="f8", name="f8")
                nc.vector.max(f8[:], m8_in)
                # dist_j/3 = sqrt((q2 - 2 t_j)/9); host passes q2/9
                d3 = smp.tile([128, K_NN], F32, tag="d3", name="d3")
                nc.scalar.activation(
                    d3[:],
                    f8[:, 0:K_NN],
                    AF.Sqrt,
                    bias=q2_sb[:, qt : qt + 1],
                    scale=-2.0 / 9.0,
                )
                nc.vector.reduce_sum(oods[qt][:], d3[:], axis=AX.X)
                if qt == 0:
                    # boundary block: gather it across the pair ASAP so
                    # the collective hides under the remaining tails
                    nc.sync.dma_start(cc_in[:], oods[0][:])
                    nc.gpsimd.collective_compute(
                        "AllGather",
                        mybir.AluOpType.bypass,
                        replica_groups=[[0, 1], [2, 3], [4, 5], [6, 7]],
                        ins=[cc_in.opt()],
                        outs=[cc_out.opt()],
                    )
                nc.sync.dma_start(
                    scratch.rearrange("(q p) -> p q", p=128)[:, qt : qt + 1],
                    oods[qt][:],
                )

            for g in range(N_G):
                for qt in range(N_QT):
                    off, t8, actb, wid = layouts[qt]
                    acts = [
                        h for h in range(UPG)
                        if (g * UPG + h) not in DVE_UNITS
                    ]
                    ev = None
                    if acts:
                        ev = evp.tile([128, 3, UCOLS], BF16, tag="ev", name="ev")
                    for h in range(UPG):
                        u = g * UPG + h
                        ps = pp.tile([128, 2, 512], F32, tag="ps", name="ps")
                        for j in range(PASSES):
                            for bk in range(2):
                                c0 = h * UCOLS + bk * 500
                                nc.tensor.matmul(
                                    ps[:, bk, 0:500],
                                    q_sb[:, j, qt],
                                    db_tiles[(g, j)][:, :, c0 : c0 + 500],
                                    start=(j == 0),
                                    stop=(j == PASSES - 1),
                                    perf_mode=DR,
                                )
                        if u in DVE_UNITS:
                            nc.vector.tensor_reduce(
                                strips[qt][:, off[u] : off[u] + DSTRIP],
                                ps[:, :, 0:500].rearrange(
                                    "p b (w k) -> p b w k", k=DWIN
                                ),
                                axis=AX.X,
                                op=MAX,
                            )
                        else:
                            nc.scalar.activation(
                                ev[:, acts.index(h), :], ps[:, :, 0:500], AF.Copy
                            )
                    if acts:
                        nr = len(acts)
                        t5 = t5p.tile([128, 3, 500], BF16, tag="t5", name="t5")
                        nc.vector.tensor_max(
                            t5[:, 0:nr, :],
                            ev[:, 0:nr, 0:500],
                            ev[:, 0:nr, 500:1000],
                        )
                        a0 = _TMP_OFF[g * UPG + acts[0]]
                        nc.vector.tensor_max(
                            tmp250[qt][:, a0 : a0 + nr * 250].rearrange(
                                "p (a c) -> p a c", c=250
                            ),
                            t5[:, 0:nr, 0:250],
                            t5[:, 0:nr, 250:500],
                        )
                    if qt == 0 and g == 3:
                        # fold groups 0-3 into the t8 slot so group 4's
                        # qt_end only scans the tail
                        na = len(ACT_A)
                        tm = tmp250[0][:].rearrange("p (a c) -> p a c", c=250)
                        nc.vector.tensor_max(
                            strips[0][
                                :, off[ACT_A[0]] : off[ACT_A[0]] + na * ASTRIP
                            ].rearrange("p (a c) -> p a c", c=ASTRIP),
                            tm[:, 0:na, 0:125],
                            tm[:, 0:na, 125:250],
                        )
                        nc.vector.max(
                            strips[0][:, t8 : t8 + 8], strips[0][:, 0:W_A]
                        )
                    if g == N_G - 1:
                        qt_end(qt)
                        if qt == 0:
                            # own ood rows can head for the upsample
                            # before the collective lands
                            pass

            # ood_hT[c, j]: j 0..15 own rows (local order), 16..23 the two
            # gathered boundary blocks in rank order
            nc.sync.dma_start(
                ood_hT[:, 0:16], scratch.rearrange("(r c) -> c r", c=W)
            )
            ood_bf = sp.tile([W, NCOL], BF16)
            nc.scalar.activation(ood_bf[:, 0:16], ood_hT[:, 0:16], AF.Copy)

            # P1[j, ow] = sum_c ood_hT[c, j] * A_c[c, ow]  (bf16, 1
            # cycle/row), split into own rows (ready early) + gathered rows
            p1a = pp.tile([16, OUT_W], F32, tag="ps", name="p1a")
            p1a_sb = sp.tile([16, OUT_W], BF16)
            nc.tensor.matmul(
                p1a[:], ood_bf[:, 0:16], ac_sb[:], start=True, stop=True,
            )
            nc.scalar.activation(p1a_sb[:], p1a[:], AF.Copy)
            p2 = [
                pp.tile([128, OUT_W], F32, tag="ps", name=f"p2_{m}")
                for m in range(2)
            ]
            for m in range(2):
                nc.tensor.matmul(
                    p2[m][:],
                    art_a[:, m * 128 : (m + 1) * 128],
                    p1a_sb[:],
                    start=True,
                    stop=False,
                )

            nc.sync.dma_start(
                ood_hT[:, 16:NCOL],
                cc_out.rearrange("(b r c) -> c (b r)", b=2, c=W),
            )
            nc.scalar.activation(ood_bf[:, 16:NCOL], ood_hT[:, 16:NCOL], AF.Copy)
            p1b = pp.tile([8, OUT_W], F32, tag="ps", name="p1b")
            p1b_sb = sp.tile([8, OUT_W], BF16)
            nc.tensor.matmul(
                p1b[:], ood_bf[:, 16:NCOL], ac_sb[:], start=True, stop=True,
            )
            nc.scalar.activation(p1b_sb[:], p1b[:], AF.Copy)
            o_sb = sp.tile([128, 2, OUT_W], F32)
            for m in range(2):
                nc.tensor.matmul(
                    p2[m][:],
                    art_b[:, m * 128 : (m + 1) * 128],
                    p1b_sb[:],
                    start=False,
                    stop=True,
                )
                nc.scalar.activation(o_sb[:, m, :], p2[m][:], AF.Copy)
            nc.sync.dma_start(
                out.rearrange("(m p) c -> p m c", m=2), o_sb[:]
            )

    nc.compile()
    return nc


def _bilinear_matrix(out_size: int, in_size: int) -> np.ndarray:
    """Half-pixel (align_corners=False) bilinear interpolation matrix
    [out_size, in_size]; edge-clamped, equivalent to jax.image.resize
    'bilinear' for integer upsampling."""
    A = np.zeros((out_size, in_size), dtype=np.float64)
    scale = in_size / out_size
    for i in range(out_size):
        s = (i + 0.5) * scale - 0.5
        j0 = int(np.floor(s))
        w = s - j0
        A[i, min(max(j0, 0), in_size - 1)] += 1.0 - w
        A[i, min(max(j0 + 1, 0), in_size - 1)] += w
    return A.astype(np.float32)


_NC_CACHE = None


def _get_nc():
    global _NC_CACHE
    if _NC_CACHE is None:
        _NC_CACHE = _build_program()
    return _NC_CACHE


def make_in_maps(embeddings: np.ndarray, database: np.ndarray):
    embeddings = np.asarray(embeddings, dtype=np.float32)
    database = np.asarray(database, dtype=np.float32)

    # db fp8 layout: rows 0..765 = dims, rows 766/767 = -||x||^2/2 in
    # split fp8 with query-side scales (2, 1)
    r = -0.5 * np.einsum("nd,nd->n", database, database)
    slotA = (r / 2.0).astype(E4M3)
    slotB = (r - 2.0 * slotA.astype(np.float32)).astype(E4M3)
    dbx = np.empty((D, N), dtype=E4M3)
    dbx[0:DREAL] = np.ascontiguousarray(database.T[0:DREAL]).astype(E4M3)
    dbx[DREAL] = slotA
    dbx[DREAL + 1] = slotB

    q_all = embeddings.transpose(0, 2, 3, 1).reshape(B, H * W, D)
    Ac = _bilinear_matrix(OUT_W, W)                      # [512, 32]
    Ar = _bilinear_matrix(OUT_H, H)                      # [512, 32]
    # the two gathered blocks in cc_out rank order: pair-core tile 0 rows
    cc_rows = [12, 13, 14, 15, 16, 17, 18, 19]

    in_maps = []
    for c in range(N_CORES):
        b, half = divmod(c, 2)
        blocks = TILE_BLOCKS[half]
        own_rows = [16 * half + 4 * blk + r_ for blk in blocks for r_ in range(4)]

        # queries in local-tile order
        q = np.concatenate(
            [
                q_all[b, (16 * half + 4 * blk) * W : (16 * half + 4 * blk + 4) * W]
                for blk in blocks
            ]
        )                                                # [512, 768]
        Qx = np.empty((D, QPC), dtype=E4M3)
        Qx[0:DREAL] = np.ascontiguousarray(q.T[0:DREAL]).astype(E4M3)
        Qx[DREAL] = 2.0
        Qx[DREAL + 1] = 1.0
        # device layout qx[p, j, qt, i, m] = Qx[256j + 128i + p, 128qt + m]
        qxb = np.ascontiguousarray(
            Qx.reshape(PASSES, 2, 128, N_QT, 128).transpose(2, 0, 3, 1, 4)
        )
        q2 = np.einsum("qd,qd->q", q, q) / 9.0
        q2 = np.ascontiguousarray(q2.reshape(N_QT, 128).T.astype(np.float32))

        # interpolation rows matching ood_hT's column order
        Arh = Ar[half * OROWS : (half + 1) * OROWS]      # [256, 32]
        art = np.zeros((NCOL, OROWS), dtype=np.float32)
        for j, row in enumerate(own_rows):
            art[j] = Arh[:, row]
        for j, row in enumerate(cc_rows):
            if row not in own_rows:
                art[16 + j] = Arh[:, row]
        in_maps.append(
            {
                "dbx": dbx,
                "qx": qxb,
                "q2": q2,
                "art": art.astype(ml_dtypes.bfloat16),
                "ac": np.ascontiguousarray(Ac.T).astype(ml_dtypes.bfloat16),
            }
        )
    return in_maps


def run_device(in_maps, **kwargs):
    nc = _get_nc()
    return bass_utils.run_bass_kernel_spmd(
        nc, in_maps, core_ids=list(range(N_CORES)), **kwargs
    )


def kernel(embeddings, database, k, out_h, out_w):
    assert int(k) == K_NN and int(out_h) == OUT_H and int(out_w) == OUT_W
    in_maps = make_in_maps(np.asarray(embeddings), np.asarray(database))
    res = run_device(in_maps)
    out = np.empty((B, 1, OUT_H, OUT_W), dtype=np.float32)
    for c in range(N_CORES):
        b, half = divmod(c, 2)
        out[b, 0, half * OROWS : (half + 1) * OROWS] = res.results[c]["out"]
    return out
